# revision 26
# baseline (speedup 1.0000x reference)
"""Trainium2 Bass kernel for nn_MultiHeadedAttention (B=2,S=2048,D=1024,H=16).

Sharding: tensor-parallel over heads — 2 heads per core x 8 cores.
Each core computes its 2 heads' attention and a partial output projection
(y_partial [B*S, D] bf16); the host sums the 8 partials and adds bo.

v2 design (vs baseline): QC=512 attention chunks with paired score PSUM
tiles [128,2,512] so the 8 PSUM banks split 4(scores)+2(PV acc)+2(misc)
with no bank contention; exp runs on [128,1024] pair tiles; QK matmuls for
the two heads are emitted adjacently (row-tiled (0,0)/(64,0)) so they can
co-execute; all HBM inputs are host-pretiled so every DMA is a 1-2 MB
fully-contiguous transfer; y is written bf16.

Device pipeline per core (matmul operands bf16, fp32 PSUM accumulate):
  qT/kT = W @ xT (feature-major), v = x @ WvT (token-major via PE transpose)
  S^T[k,q] = K @ Q^T (2 heads row-tiled, contraction dk=64)
  p = exp(S^T) * em   (em = exp(bias)*(1-mask), host-precomputed, bf16)
  OT = [V|1]^T @ p (per head, M=65; row 64 = softmax denominator)
  OTn = OT * recip(d) -> y_partial = OTn^T @ WoT
"""

import numpy as np
import ml_dtypes

import concourse.bass as bass
import concourse.tile as tile
from concourse import bacc, mybir
from concourse.bass import ts
from concourse.bass_utils import run_bass_kernel_spmd
from concourse.masks import make_identity

BF16 = ml_dtypes.bfloat16

B, S, D, H = 2, 2048, 1024, 16
N_CORES = 8
HC = H // N_CORES          # heads per core = 2
DK = D // H                # 64
DKC = HC * DK              # head dims per core = 128
P = 128
T = B * S                  # 4096 tokens
KO = D // P                # 8 feature k-subtiles
TC = 512                   # token chunk for projections
QC = 512                   # q chunk for attention phase
NKS = S // P               # 16 k-subtiles per batch
NPAIR = NKS // 2           # 8 ks-pairs per batch
NQC = S // QC              # 4 q-chunks per batch
NCH = T // TC              # 8 projection chunks

bf = mybir.dt.bfloat16
f32 = mybir.dt.float32


class _Ctx:
    pass


def _proj_chunk(nc, g, c):
    """Emit projection work for token chunk c (512 tokens).

    v is computed feature-major (dense N=512 matmuls), then PE-transposed
    into the token-major [tok, dk] layout PV needs; k/q stay feature-major.
    """
    io = g.io
    xv = g.stream_pool.tile([P, KO, TC], bf, tag="xv", bufs=3, name=f"xv{c}")
    nc.sync.dma_start(xv[:], io["xvT"][c])
    # v computed token-major directly: lhsT = x chunk (tokens as columns)
    for tt in range(TC // P):
        ps_t = g.psum_pool.tile([P, P], f32, tag="ps", bufs=3,
                                name=f"pst{c}_{tt}")
        for ko in range(KO):
            nc.tensor.matmul(
                ps_t[:], xv[:, ko, ts(tt, P)], g.wv_sb[:, ko, :],
                start=(ko == 0), stop=(ko == KO - 1),
            )
        vt_i = c * (TC // P) + tt
        # v layout per 128-token tile: [vA(64) | 1 | vB(64) | 1]
        nc.vector.tensor_copy(g.v_sb[:, vt_i, 0:DK], ps_t[:, 0:DK])
        nc.vector.tensor_copy(g.v_sb[:, vt_i, DK + 1:DKC + 1], ps_t[:, DK:DKC])

    xk = g.stream_pool.tile([P, KO, TC], bf, tag="xk", bufs=3, name=f"xk{c}")
    nc.sync.dma_start(xk[:], io["xkT"][c])
    ps_k = g.psum_pool.tile([P, TC], f32, tag="ps", bufs=3, name=f"psk{c}")
    for ko in range(KO):
        nc.tensor.matmul(
            ps_k[:], g.wk_sb[:, ko, :], xk[:, ko, :],
            start=(ko == 0), stop=(ko == KO - 1),
        )
    nc.vector.tensor_copy(g.kT_sb[:, ts(c, TC)], ps_k[:])

    xq = g.stream_pool.tile([P, KO, TC], bf, tag="xq", bufs=3, name=f"xq{c}")
    nc.sync.dma_start(xq[:], io["xqT"][c])
    ps_q = g.psum_pool.tile([P, TC], f32, tag="ps", bufs=3, name=f"psq{c}")
    for ko in range(KO):
        nc.tensor.matmul(
            ps_q[:], g.wq_sb[:, ko, :], xq[:, ko, :],
            start=(ko == 0), stop=(ko == KO - 1),
        )
    nc.vector.tensor_copy(g.qT_sb[:, ts(c, TC)], ps_q[:])


def _attention_chunk(nc, g, b, qc, fillers):
    """Emit attention for (batch b, q-chunk qc of 512). `fillers` is a list
    of callables emitted early in the pair loop as PE filler (deferred
    y-projections of an earlier chunk). PV is software-pipelined one
    ks-pair behind QK so the in-order PE queue never waits on exp/mul."""
    io = g.io
    qs = b * S + qc * QC
    # em tiles for this (b, qc): one per head, [ki, ks, q] = 2 MB contiguous
    emA = g.em_pool.tile([P, NKS, QC], bf, tag="em", bufs=4, name=f"emA{b}_{qc}")
    nc.sync.dma_start(emA[:], io["em"][b, 0, qc])
    emB = g.em_pool.tile([P, NKS, QC], bf, tag="em", bufs=4, name=f"emB{b}_{qc}")
    nc.sync.dma_start(emB[:], io["em"][b, 1, qc])

    ps_oA = g.psum_pool.tile([DK + 1, QC], f32, tag="ps_oA", bufs=1,
                             name=f"psoA{b}_{qc}")
    ps_oB = g.psum_pool.tile([DK + 1, QC], f32, tag="ps_oB", bufs=1,
                             name=f"psoB{b}_{qc}")

    def emit_qk(j):
        ps_sA = g.psum_pool.tile([P, 2, QC], f32, tag="ps", bufs=3,
                                 name=f"pssA{b}_{qc}_{j}")
        ps_sB = g.psum_pool.tile([P, 2, QC], f32, tag="ps", bufs=3,
                                 name=f"pssB{b}_{qc}_{j}")
        # head A's pair first so exp_A can start two matmuls earlier
        for lo, hi, pst in ((0, DK, ps_sA), (DK, P, ps_sB)):
            for jj in range(2):
                ks = 2 * j + jj
                kslice = b * S + ks * P
                nc.tensor.matmul(
                    pst[:, jj, :],
                    g.kT_sb[lo:hi, kslice:kslice + P],
                    g.qT_sb[lo:hi, qs:qs + QC],
                    start=True, stop=True,
                )
        eA = g.work_pool.tile([P, 2, QC], bf, tag="e", bufs=3,
                              name=f"eA{b}_{qc}_{j}")
        nc.scalar.activation(eA[:], ps_sA[:], mybir.ActivationFunctionType.Exp)
        pA = g.work_pool.tile([P, 2, QC], bf, tag="p", bufs=3,
                              name=f"pA{b}_{qc}_{j}")
        nc.vector.tensor_mul(pA[:], eA[:], emA[:, 2 * j:2 * j + 2, :])
        eB = g.work_pool.tile([P, 2, QC], bf, tag="e", bufs=3,
                              name=f"eB{b}_{qc}_{j}")
        nc.scalar.activation(eB[:], ps_sB[:], mybir.ActivationFunctionType.Exp)
        pB = g.work_pool.tile([P, 2, QC], bf, tag="p", bufs=3,
                              name=f"pB{b}_{qc}_{j}")
        nc.vector.tensor_mul(pB[:], eB[:], emB[:, 2 * j:2 * j + 2, :])
        return pA, pB

    def emit_pv(j, pA, pB):
        first = j == 0
        last = j == NPAIR - 1
        for jj in range(2):
            ks = 2 * j + jj
            vt = (b * S + ks * P) // P
            nc.tensor.matmul(
                ps_oA[:], g.v_sb[:, vt, 0:DK + 1], pA[:, jj, :],
                start=(first and jj == 0), stop=(last and jj == 1),
            )
            nc.tensor.matmul(
                ps_oB[:], g.v_sb[:, vt, DK + 1:P + 2], pB[:, jj, :],
                start=(first and jj == 0), stop=(last and jj == 1),
            )

    prev = None
    for j in range(NPAIR):
        p_cur = emit_qk(j)
        if j == 1 and g.pre_chunk is not None:
            g.pre_chunk()
            g.pre_chunk = None
        if fillers and j >= 2:
            fillers.pop(0)()
        if prev is not None:
            emit_pv(j - 1, *prev)
        prev = p_cur
    emit_pv(NPAIR - 1, *prev)
    # normalize: OTn_h = OT_h * (1/d_h); 1/d row broadcast to 64 partitions
    # via a K=1 matmul (operands at partition base 64... kept at base 0).
    ot_sb = g.work_pool.tile([P, QC], bf, tag="ot", bufs=3, name=f"ot{b}_{qc}")
    otB_t = g.work_pool.tile([DK, QC], bf, tag="otB", bufs=3, name=f"otB{b}_{qc}")
    # normalize phase 1 (DVE only, inline): 1/(d|OT) per head
    r65bs = []
    for hi, ps_oX in enumerate((ps_oA, ps_oB)):
        r65 = g.work_pool.tile([DK + 1, QC], f32, tag="r65", bufs=2,
                               name=f"r65_{b}_{qc}_{hi}")
        nc.vector.reciprocal_approx_fast(r65[:], ps_oX[:])
        r65b = g.work_pool.tile([DK + 1, QC], bf, tag="r65b", bufs=4,
                                name=f"r65b_{b}_{qc}_{hi}")
        nc.vector.tensor_copy(r65b[:], r65[:])
        r65bs.append(r65b)

    def phase2():
        # broadcast 1/d + apply + move head B rows — deferred into the next
        # chunk so the PE queue is never head-of-line blocked on the DVE
        # recip chain at the chunk boundary.
        for hi, (ps_oX, r65b, out_ap) in enumerate(
                ((ps_oA, r65bs[0], ot_sb[0:DK, :]),
                 (ps_oB, r65bs[1], otB_t[:]))):
            ps_r = g.psum_pool.tile([DK, QC], f32, tag="ps", bufs=3,
                                    name=f"psr{b}_{qc}_{hi}")
            nc.tensor.matmul(
                ps_r[:], g.dsel_sb[DK:DK + 1, 0:DK], r65b[DK:DK + 1, :],
                start=True, stop=True,
            )
            rb_sb = g.work_pool.tile([DK, QC], f32, tag="rbs", bufs=2,
                                     name=f"rbs{b}_{qc}_{hi}")
            nc.vector.tensor_copy(rb_sb[:], ps_r[:])
            nc.vector.tensor_mul(out_ap, ps_oX[0:DK, :], rb_sb[:])
        nc.gpsimd.dma_start(ot_sb[DK:P, :], otB_t[:])

    return ot_sb, phase2


def _make_yproj_fillers(nc, g, b, qc, ot_sb):
    """Build deferred y-projection emitters for chunk (b, qc): 4 token
    subtiles x 2 D-halves, each a [128,512] matmul + copy + DMA out."""
    io = g.io
    qs = b * S + qc * QC

    def emit(qsub):
        y_sb = g.work_pool.tile([P, D], bf, tag="ysb", bufs=2,
                                name=f"ysb{b}_{qc}_{qsub}")
        ps_y = g.psum_pool.tile([P, 2, 512], f32, tag="ps", bufs=3,
                                name=f"psy{b}_{qc}_{qsub}")
        for ch in range(D // 512):
            nc.tensor.matmul(
                ps_y[:, ch, :], ot_sb[:, ts(qsub, P)], g.wo_sb[:, ts(ch, 512)],
                start=True, stop=True,
            )
        nc.vector.tensor_copy(y_sb[:, 0:512], ps_y[:, 0, :])
        nc.scalar.copy(y_sb[:, 512:D], ps_y[:, 1, :])
        nc.sync.dma_start(io["y"][qs + qsub * P:qs + (qsub + 1) * P, :], y_sb[:])

    return [lambda qsub=qsub: emit(qsub) for qsub in range(QC // P)]


def _build_body(nc, tc, io):
    from contextlib import ExitStack
    ctx = ExitStack()
    g = _Ctx()
    g.io = io
    g.const_pool = ctx.enter_context(tc.tile_pool(name="const", bufs=1))
    g.stream_pool = ctx.enter_context(tc.tile_pool(name="stream", bufs=2))
    g.em_pool = ctx.enter_context(tc.tile_pool(name="em", bufs=4))
    g.work_pool = ctx.enter_context(tc.tile_pool(name="work", bufs=2))
    g.psum_pool = ctx.enter_context(tc.tile_pool(name="psum", bufs=2, space="PSUM"))

    # ---- persistent SBUF tensors ----
    g.wq_sb = g.const_pool.tile([P, KO, DKC], bf, tag="wq", name="wq_sb")
    nc.sync.dma_start(g.wq_sb[:], io["wqT"])
    g.wk_sb = g.const_pool.tile([P, KO, DKC], bf, tag="wk", name="wk_sb")
    nc.sync.dma_start(g.wk_sb[:], io["wkT"])
    g.wv_sb = g.const_pool.tile([P, KO, DKC], bf, tag="wv", name="wv_sb")
    nc.sync.dma_start(g.wv_sb[:], io["wvT"])
    g.wo_sb = g.const_pool.tile([P, D], bf, tag="wo", name="wo_sb")
    nc.sync.dma_start(g.wo_sb[:], io["woT"])
    g.ident_sb = g.const_pool.tile([P, P], bf, tag="ident", name="ident_sb")
    make_identity(nc, g.ident_sb[:])
    # dsel[k,m] = 1 iff k==64: selector that broadcasts r65b's row 64
    g.dsel_sb = g.const_pool.tile([P, DK + 1], bf, tag="dsel", name="dsel_sb")
    nc.vector.memset(g.dsel_sb[:], 0.0)
    nc.vector.memset(g.dsel_sb[DK:DK + 1, :], 1.0)

    g.qT_sb = g.const_pool.tile([P, T], bf, tag="qT", name="qT_sb")
    g.kT_sb = g.const_pool.tile([P, T], bf, tag="kT", name="kT_sb")
    # v layout per 128-token tile: [vA(64) | 1 | vB(64) | 1] for ones-aug PV.
    # Full-tile memset; the v copies overwrite all but the ones-columns.
    g.v_sb = g.const_pool.tile([P, T // P, DKC + 2], bf, tag="v", name="v_sb")
    nc.vector.memset(g.v_sb[:], 1.0)

    # batch-0 projections
    for c in range(4):
        _proj_chunk(nc, g, c)
    # batch-0 attention with batch-1 projections as PE filler between
    # chunks. y-projection of chunk c is emitted inside chunk c+2 (the ot
    # pool holds 3 chunks) so the filler matmuls never wait on normalize.
    g.pre_chunk = None
    pending = []
    proj_after = {(0, 0): [4], (0, 1): [5], (0, 2): [6], (0, 3): [7]}
    for b in range(B):
        for qc in range(NQC):
            fillers = pending.pop(0) if len(pending) >= 2 else []
            ot, phase2 = _attention_chunk(nc, g, b, qc, fillers)
            g.pre_chunk = phase2
            pending.append(_make_yproj_fillers(nc, g, b, qc, ot))
            for c in proj_after.get((b, qc), []):
                _proj_chunk(nc, g, c)
    if g.pre_chunk is not None:
        g.pre_chunk()
        g.pre_chunk = None
    for fillers in pending:
        for f in fillers:
            f()

    ctx.close()


def build_nc():
    nc = bacc.Bacc("TRN2", target_bir_lowering=False, debug=False,
                   num_devices=N_CORES)
    io = {
        # pretiled [chunk, ki, ko, t]: 1 MB contiguous per chunk
        "xqT": nc.dram_tensor("xqT", [NCH, P, KO, TC], bf, kind="ExternalInput").ap(),
        "xkT": nc.dram_tensor("xkT", [NCH, P, KO, TC], bf, kind="ExternalInput").ap(),
        "xvT": nc.dram_tensor("xvT", [NCH, P, KO, TC], bf, kind="ExternalInput").ap(),
        # pretiled [ki, ko, m]
        "wqT": nc.dram_tensor("wqT", [P, KO, DKC], bf, kind="ExternalInput").ap(),
        "wkT": nc.dram_tensor("wkT", [P, KO, DKC], bf, kind="ExternalInput").ap(),
        "wvT": nc.dram_tensor("wvT", [P, KO, DKC], bf, kind="ExternalInput").ap(),
        "woT": nc.dram_tensor("woT", [DKC, D], bf, kind="ExternalInput").ap(),
        # em pretiled [b, h, qc, ki, ks, q]: 2 MB contiguous per (b,h,qc)
        "em": nc.dram_tensor("em", [B, HC, NQC, P, NKS, QC], bf,
                             kind="ExternalInput").ap(),
        "y": nc.dram_tensor("y", [T, D], bf, kind="ExternalOutput").ap(),
    }
    with tile.TileContext(nc) as tc:
        _build_body(nc, tc, io)
    nc.compile()
    return nc


_NC_CACHE = None


def _get_nc():
    global _NC_CACHE
    if _NC_CACHE is None:
        _NC_CACHE = build_nc()
    return _NC_CACHE


def _pretile_x(xT):
    """[D, T] -> [chunk, ki, ko, t] contiguous."""
    return np.ascontiguousarray(
        xT.reshape(KO, P, NCH, TC).transpose(2, 1, 0, 3)
    )


def make_in_maps(query, key, value, mask, rel_pos_bias,
                 Wq, bq, Wk, bk, Wv, bv, Wo, bo):
    """Host-side sharding/preprocessing -> per-core input dicts."""
    xqT = _pretile_x(query.reshape(T, D).T.astype(BF16))
    xkT = _pretile_x(key.reshape(T, D).T.astype(BF16))
    xvT = _pretile_x(value.reshape(T, D).T.astype(BF16))

    scale = 1.0 / np.sqrt(np.float32(DK))
    maskinv = (~mask[:, 0]).astype(np.float32)          # [B, Sq, Sk]

    # bq/bk handling: scores_full = (q+bq)(k+bk)^T * scale.
    # The (q'+bq)·bk term varies only along q => softmax-invariant, dropped.
    # The bq·(k'+bk) term varies along k; fold exp(delta_k) into em when
    # bq is nonzero (needs host k-projection).
    need_delta = bool(np.any(bq))
    if need_delta:
        k_proj = key.reshape(T, D).astype(np.float32) @ Wk.T.astype(np.float32) + bk

    in_maps = []
    for c in range(N_CORES):
        hs = slice(c * DKC, (c + 1) * DKC)
        wqT = np.ascontiguousarray(
            (Wq[hs, :] * scale).T.astype(BF16).reshape(KO, P, DKC).transpose(1, 0, 2))
        wkT = np.ascontiguousarray(
            Wk[hs, :].T.astype(BF16).reshape(KO, P, DKC).transpose(1, 0, 2))
        wvT = np.ascontiguousarray(
            Wv[hs, :].T.astype(BF16).reshape(KO, P, DKC).transpose(1, 0, 2))
        woT = np.ascontiguousarray(Wo[:, hs].T.astype(BF16))
        em = np.empty((B, HC, NQC, P, NKS, QC), dtype=BF16)
        for hi in range(HC):
            h = c * HC + hi
            ebT = np.exp(rel_pos_bias[0, h].astype(np.float32)).T  # [k, q]
            for b_ in range(B):
                ebb = ebT
                if need_delta:
                    delta = scale * (
                        k_proj[b_ * S:(b_ + 1) * S, h * DK:(h + 1) * DK]
                        @ bq[h * DK:(h + 1) * DK]
                        + np.dot(bq[h * DK:(h + 1) * DK], bk[h * DK:(h + 1) * DK])
                    )  # [S] along k
                    ebb = ebT * np.exp(delta)[:, None]
                emf = (ebb * maskinv[b_].T).astype(BF16)       # [k, q]
                # [k, q] -> [qc, ki, ks, q]
                em[b_, hi] = emf.reshape(NKS, P, NQC, QC).transpose(2, 1, 0, 3)
        in_maps.append({
            "xqT": xqT, "xkT": xkT, "xvT": xvT,
            "wqT": wqT, "wkT": wkT, "wvT": wvT, "woT": woT,
            "em": em,
        })
    return in_maps


def assemble_output(results, value_bias, Wo, bo):
    out = np.zeros((T, D), np.float32)
    for r in results:
        out += r["y"].astype(np.float32)
    # exact bv contribution: softmax rows sum to 1 => attn_out += bv,
    # so y += bv @ Wo^T; plus bo.
    out += value_bias.astype(np.float32) @ Wo.T.astype(np.float32)
    out += bo.astype(np.float32)[None, :]
    return out.reshape(B, S, D)


def kernel(query, key, value, mask, rel_pos_bias,
           Wq, bq, Wk, bk, Wv, bv, Wo, bo, _run_kwargs=None):
    query = np.asarray(query); key = np.asarray(key); value = np.asarray(value)
    mask = np.asarray(mask); rel_pos_bias = np.asarray(rel_pos_bias)
    Wq = np.asarray(Wq); Wk = np.asarray(Wk); Wv = np.asarray(Wv)
    Wo = np.asarray(Wo)
    bq = np.asarray(bq); bk = np.asarray(bk); bv = np.asarray(bv)
    bo = np.asarray(bo)

    nc = _get_nc()
    in_maps = make_in_maps(query, key, value, mask, rel_pos_bias,
                           Wq, bq, Wk, bk, Wv, bv, Wo, bo)
    kw = _run_kwargs or {}
    res = run_bass_kernel_spmd(nc, in_maps, core_ids=list(range(N_CORES)), **kw)
    out = assemble_output(res.results, bv, Wo, bo)
    if _run_kwargs is not None:
        kernel._last_results = res
    return out


# revision 27
# speedup vs baseline: 1.1575x; 1.1575x over previous
"""Trainium2 Bass kernel for nn_MultiHeadedAttention (B=2,S=2048,D=1024,H=16).

Sharding: tensor-parallel over heads — 2 heads per core x 8 cores.
Each core computes its 2 heads' attention and a partial output projection
(y_partial [B*S, D] bf16); the host sums the 8 partials and adds bo.

v2 design (vs baseline): QC=512 attention chunks with paired score PSUM
tiles [128,2,512] so the 8 PSUM banks split 4(scores)+2(PV acc)+2(misc)
with no bank contention; exp runs on [128,1024] pair tiles; QK matmuls for
the two heads are emitted adjacently (row-tiled (0,0)/(64,0)) so they can
co-execute; all HBM inputs are host-pretiled so every DMA is a 1-2 MB
fully-contiguous transfer; y is written bf16.

Device pipeline per core (matmul operands bf16, fp32 PSUM accumulate):
  qT/kT = W @ xT (feature-major), v = x @ WvT (token-major via PE transpose)
  S^T[k,q] = K @ Q^T (2 heads row-tiled, contraction dk=64)
  p = exp(S^T) * em   (em = exp(bias)*(1-mask), host-precomputed, bf16)
  OT = [V|1]^T @ p (per head, M=65; row 64 = softmax denominator)
  OTn = OT * recip(d) -> y_partial = OTn^T @ WoT
"""

import numpy as np
import ml_dtypes

import concourse.bass as bass
import concourse.tile as tile
from concourse import bacc, mybir
from concourse.bass import ts
from concourse.bass_utils import run_bass_kernel_spmd
from concourse.masks import make_identity

BF16 = ml_dtypes.bfloat16

B, S, D, H = 2, 2048, 1024, 16
N_CORES = 8
HC = H // N_CORES          # heads per core = 2
DK = D // H                # 64
DKC = HC * DK              # head dims per core = 128
P = 128
T = B * S                  # 4096 tokens
KO = D // P                # 8 feature k-subtiles
TC = 512                   # token chunk for projections
QC = 512                   # q chunk for attention phase
NKS = S // P               # 16 k-subtiles per batch
NPAIR = NKS // 2           # 8 ks-pairs per batch
NQC = S // QC              # 4 q-chunks per batch
NCH = T // TC              # 8 projection chunks

bf = mybir.dt.bfloat16
f32 = mybir.dt.float32


class _Ctx:
    pass


def _proj_chunk(nc, g, c):
    """Emit projection work for token chunk c (512 tokens).

    v is computed feature-major (dense N=512 matmuls), then PE-transposed
    into the token-major [tok, dk] layout PV needs; k/q stay feature-major.
    """
    io = g.io
    xv = g.stream_pool.tile([P, KO, TC], bf, tag="xv", bufs=3, name=f"xv{c}")
    nc.sync.dma_start(xv[:], io["xvT"][c])
    # v computed token-major directly: lhsT = x chunk (tokens as columns)
    for tt in range(TC // P):
        ps_t = g.psum_pool.tile([P, P], f32, tag="ps", bufs=3,
                                name=f"pst{c}_{tt}")
        for ko in range(KO):
            nc.tensor.matmul(
                ps_t[:], xv[:, ko, ts(tt, P)], g.wv_sb[:, ko, :],
                start=(ko == 0), stop=(ko == KO - 1),
            )
        vt_i = c * (TC // P) + tt
        # v layout per 128-token tile: [vA(64) | 1 | vB(64) | 1]
        nc.vector.tensor_copy(g.v_sb[:, vt_i, 0:DK], ps_t[:, 0:DK])
        nc.vector.tensor_copy(g.v_sb[:, vt_i, DK + 1:DKC + 1], ps_t[:, DK:DKC])

    xk = g.stream_pool.tile([P, KO, TC], bf, tag="xk", bufs=3, name=f"xk{c}")
    nc.sync.dma_start(xk[:], io["xkT"][c])
    ps_k = g.psum_pool.tile([P, TC], f32, tag="ps", bufs=3, name=f"psk{c}")
    for ko in range(KO):
        nc.tensor.matmul(
            ps_k[:], g.wk_sb[:, ko, :], xk[:, ko, :],
            start=(ko == 0), stop=(ko == KO - 1),
        )
    nc.vector.tensor_copy(g.kT_sb[:, ts(c, TC)], ps_k[:])

    xq = g.stream_pool.tile([P, KO, TC], bf, tag="xq", bufs=3, name=f"xq{c}")
    nc.sync.dma_start(xq[:], io["xqT"][c])
    ps_q = g.psum_pool.tile([P, TC], f32, tag="ps", bufs=3, name=f"psq{c}")
    for ko in range(KO):
        nc.tensor.matmul(
            ps_q[:], g.wq_sb[:, ko, :], xq[:, ko, :],
            start=(ko == 0), stop=(ko == KO - 1),
        )
    nc.vector.tensor_copy(g.qT_sb[:, ts(c, TC)], ps_q[:])


def _attention_chunk(nc, g, b, qc, fillers, mid_hooks=None):
    """Emit attention for (batch b, q-chunk qc of 512). `fillers` is a list
    of callables emitted early in the pair loop as PE filler (deferred
    y-projections of an earlier chunk). PV is software-pipelined one
    ks-pair behind QK so the in-order PE queue never waits on exp/mul."""
    io = g.io
    qs = b * S + qc * QC
    # em tiles for this (b, qc): one per head, [ki, ks, q] = 2 MB contiguous
    emA = g.em_pool.tile([P, NKS, QC], bf, tag="em", bufs=4, name=f"emA{b}_{qc}")
    nc.sync.dma_start(emA[:], io["em"][b, 0, qc])
    emB = g.em_pool.tile([P, NKS, QC], bf, tag="em", bufs=4, name=f"emB{b}_{qc}")
    nc.sync.dma_start(emB[:], io["em"][b, 1, qc])

    ps_oA = g.psum_pool.tile([DK + 1, QC], f32, tag="ps_oA", bufs=1,
                             name=f"psoA{b}_{qc}")
    ps_oB = g.psum_pool.tile([DK + 1, QC], f32, tag="ps_oB", bufs=1,
                             name=f"psoB{b}_{qc}")

    def emit_qk(j):
        ps_sA = g.psum_pool.tile([P, 2, QC], f32, tag="ps", bufs=3,
                                 name=f"pssA{b}_{qc}_{j}")
        ps_sB = g.psum_pool.tile([P, 2, QC], f32, tag="ps", bufs=3,
                                 name=f"pssB{b}_{qc}_{j}")
        # head A's pair first so exp_A can start two matmuls earlier
        for lo, hi, pst in ((0, DK, ps_sA), (DK, P, ps_sB)):
            for jj in range(2):
                ks = 2 * j + jj
                kslice = b * S + ks * P
                nc.tensor.matmul(
                    pst[:, jj, :],
                    g.kT_sb[lo:hi, kslice:kslice + P],
                    g.qT_sb[lo:hi, qs:qs + QC],
                    start=True, stop=True,
                )
        eA = g.work_pool.tile([P, 2, QC], bf, tag="e", bufs=3,
                              name=f"eA{b}_{qc}_{j}")
        nc.scalar.activation(eA[:], ps_sA[:], mybir.ActivationFunctionType.Exp)
        pA = g.work_pool.tile([P, 2, QC], bf, tag="p", bufs=3,
                              name=f"pA{b}_{qc}_{j}")
        nc.vector.tensor_mul(pA[:], eA[:], emA[:, 2 * j:2 * j + 2, :])
        eB = g.work_pool.tile([P, 2, QC], bf, tag="e", bufs=3,
                              name=f"eB{b}_{qc}_{j}")
        nc.scalar.activation(eB[:], ps_sB[:], mybir.ActivationFunctionType.Exp)
        pB = g.work_pool.tile([P, 2, QC], bf, tag="p", bufs=3,
                              name=f"pB{b}_{qc}_{j}")
        nc.vector.tensor_mul(pB[:], eB[:], emB[:, 2 * j:2 * j + 2, :])
        return pA, pB

    def emit_pv(j, pA, pB):
        first = j == 0
        last = j == NPAIR - 1
        for jj in range(2):
            ks = 2 * j + jj
            vt = (b * S + ks * P) // P
            nc.tensor.matmul(
                ps_oA[:], g.v_sb[:, vt, 0:DK + 1], pA[:, jj, :],
                start=(first and jj == 0), stop=(last and jj == 1),
            )
            nc.tensor.matmul(
                ps_oB[:], g.v_sb[:, vt, DK + 1:P + 2], pB[:, jj, :],
                start=(first and jj == 0), stop=(last and jj == 1),
            )

    prev = None
    for j in range(NPAIR):
        p_cur = emit_qk(j)
        if mid_hooks and j in mid_hooks:
            mid_hooks[j]()
        if j == 1 and g.pre_chunk is not None:
            g.pre_chunk()
            g.pre_chunk = None
        if fillers and j >= 2:
            fillers.pop(0)()
        if prev is not None:
            emit_pv(j - 1, *prev)
        prev = p_cur
    emit_pv(NPAIR - 1, *prev)
    # normalize: OTn_h = OT_h * (1/d_h); 1/d row broadcast to 64 partitions
    # via a K=1 matmul (operands at partition base 64... kept at base 0).
    ot_sb = g.work_pool.tile([P, QC], bf, tag="ot", bufs=3, name=f"ot{b}_{qc}")
    otB_t = g.work_pool.tile([DK, QC], bf, tag="otB", bufs=3, name=f"otB{b}_{qc}")
    # normalize phase 1 (DVE only, inline): 1/(d|OT) per head
    r65bs = []
    for hi, ps_oX in enumerate((ps_oA, ps_oB)):
        r65 = g.work_pool.tile([DK + 1, QC], f32, tag="r65", bufs=2,
                               name=f"r65_{b}_{qc}_{hi}")
        nc.vector.reciprocal_approx_fast(r65[:], ps_oX[:])
        r65b = g.work_pool.tile([DK + 1, QC], bf, tag="r65b", bufs=4,
                                name=f"r65b_{b}_{qc}_{hi}")
        nc.vector.tensor_copy(r65b[:], r65[:])
        r65bs.append(r65b)

    def phase2():
        # broadcast 1/d + apply + move head B rows — deferred into the next
        # chunk so the PE queue is never head-of-line blocked on the DVE
        # recip chain at the chunk boundary.
        for hi, (ps_oX, r65b, out_ap) in enumerate(
                ((ps_oA, r65bs[0], ot_sb[0:DK, :]),
                 (ps_oB, r65bs[1], otB_t[:]))):
            ps_r = g.psum_pool.tile([DK, QC], f32, tag="ps", bufs=3,
                                    name=f"psr{b}_{qc}_{hi}")
            nc.tensor.matmul(
                ps_r[:], g.dsel_sb[DK:DK + 1, 0:DK], r65b[DK:DK + 1, :],
                start=True, stop=True,
            )
            rb_sb = g.work_pool.tile([DK, QC], f32, tag="rbs", bufs=2,
                                     name=f"rbs{b}_{qc}_{hi}")
            nc.vector.tensor_copy(rb_sb[:], ps_r[:])
            nc.vector.tensor_mul(out_ap, ps_oX[0:DK, :], rb_sb[:])
        nc.gpsimd.dma_start(ot_sb[DK:P, :], otB_t[:])

    return ot_sb, phase2


def _make_yproj_fillers(nc, g, b, qc, ot_sb):
    """Build deferred y-projection emitters for chunk (b, qc): 4 token
    subtiles x 2 D-halves, each a [128,512] matmul + copy + DMA out."""
    io = g.io
    qs = b * S + qc * QC

    def emit(qsub):
        y_sb = g.work_pool.tile([P, D], bf, tag="ysb", bufs=2,
                                name=f"ysb{b}_{qc}_{qsub}")
        ps_y = g.psum_pool.tile([P, 2, 512], f32, tag="ps", bufs=3,
                                name=f"psy{b}_{qc}_{qsub}")
        for ch in range(D // 512):
            nc.tensor.matmul(
                ps_y[:, ch, :], ot_sb[:, ts(qsub, P)], g.wo_sb[:, ts(ch, 512)],
                start=True, stop=True,
            )
        nc.vector.tensor_copy(y_sb[:, 0:512], ps_y[:, 0, :])
        nc.scalar.copy(y_sb[:, 512:D], ps_y[:, 1, :])
        nc.sync.dma_start(io["y"][qs + qsub * P:qs + (qsub + 1) * P, :], y_sb[:])

    return [lambda qsub=qsub: emit(qsub) for qsub in range(QC // P)]


def _build_body(nc, tc, io):
    from contextlib import ExitStack
    ctx = ExitStack()
    g = _Ctx()
    g.io = io
    g.const_pool = ctx.enter_context(tc.tile_pool(name="const", bufs=1))
    g.stream_pool = ctx.enter_context(tc.tile_pool(name="stream", bufs=2))
    g.em_pool = ctx.enter_context(tc.tile_pool(name="em", bufs=4))
    g.work_pool = ctx.enter_context(tc.tile_pool(name="work", bufs=2))
    g.psum_pool = ctx.enter_context(tc.tile_pool(name="psum", bufs=2, space="PSUM"))

    # ---- persistent SBUF tensors ----
    g.wq_sb = g.const_pool.tile([P, KO, DKC], bf, tag="wq", name="wq_sb")
    nc.sync.dma_start(g.wq_sb[:], io["wqT"])
    g.wk_sb = g.const_pool.tile([P, KO, DKC], bf, tag="wk", name="wk_sb")
    nc.sync.dma_start(g.wk_sb[:], io["wkT"])
    g.wv_sb = g.const_pool.tile([P, KO, DKC], bf, tag="wv", name="wv_sb")
    nc.sync.dma_start(g.wv_sb[:], io["wvT"])
    g.wo_sb = g.const_pool.tile([P, D], bf, tag="wo", name="wo_sb")
    nc.sync.dma_start(g.wo_sb[:], io["woT"])
    g.ident_sb = g.const_pool.tile([P, P], bf, tag="ident", name="ident_sb")
    make_identity(nc, g.ident_sb[:])
    # dsel[k,m] = 1 iff k==64: selector that broadcasts r65b's row 64
    g.dsel_sb = g.const_pool.tile([P, DK + 1], bf, tag="dsel", name="dsel_sb")
    nc.vector.memset(g.dsel_sb[:], 0.0)
    nc.vector.memset(g.dsel_sb[DK:DK + 1, :], 1.0)

    g.qT_sb = g.const_pool.tile([P, T], bf, tag="qT", name="qT_sb")
    g.kT_sb = g.const_pool.tile([P, T], bf, tag="kT", name="kT_sb")
    # v layout per 128-token tile: [vA(64) | 1 | vB(64) | 1] for ones-aug PV.
    # Full-tile memset; the v copies overwrite all but the ones-columns.
    g.v_sb = g.const_pool.tile([P, T // P, DKC + 2], bf, tag="v", name="v_sb")
    nc.vector.memset(g.v_sb[:], 1.0)

    # batch-0 attention starts right after projection chunk 0: QK pair j
    # only needs k-tokens from projection chunk j//2, so chunks 1-3 are
    # emitted inside the first attention chunk's pair loop (the PE stream
    # stays dense through the DMA-bound projection phase). Batch-1
    # projections fill between later chunks. y-projection of chunk c is
    # emitted inside chunk c+2 (the ot pool holds 3 chunks) so the filler
    # matmuls never wait on normalize.
    g.pre_chunk = None
    pending = []
    _proj_chunk(nc, g, 0)
    mid0 = {1: lambda: _proj_chunk(nc, g, 1),
            3: lambda: _proj_chunk(nc, g, 2),
            5: lambda: _proj_chunk(nc, g, 3)}
    proj_after = {(0, 0): [4], (0, 1): [5], (0, 2): [6], (0, 3): [7]}
    for b in range(B):
        for qc in range(NQC):
            fillers = pending.pop(0) if len(pending) >= 2 else []
            mh = mid0 if (b, qc) == (0, 0) else None
            ot, phase2 = _attention_chunk(nc, g, b, qc, fillers, mh)
            g.pre_chunk = phase2
            pending.append(_make_yproj_fillers(nc, g, b, qc, ot))
            for c in proj_after.get((b, qc), []):
                _proj_chunk(nc, g, c)
    if g.pre_chunk is not None:
        g.pre_chunk()
        g.pre_chunk = None
    for fillers in pending:
        for f in fillers:
            f()

    ctx.close()


def build_nc():
    nc = bacc.Bacc("TRN2", target_bir_lowering=False, debug=False,
                   num_devices=N_CORES)
    io = {
        # pretiled [chunk, ki, ko, t]: 1 MB contiguous per chunk
        "xqT": nc.dram_tensor("xqT", [NCH, P, KO, TC], bf, kind="ExternalInput").ap(),
        "xkT": nc.dram_tensor("xkT", [NCH, P, KO, TC], bf, kind="ExternalInput").ap(),
        "xvT": nc.dram_tensor("xvT", [NCH, P, KO, TC], bf, kind="ExternalInput").ap(),
        # pretiled [ki, ko, m]
        "wqT": nc.dram_tensor("wqT", [P, KO, DKC], bf, kind="ExternalInput").ap(),
        "wkT": nc.dram_tensor("wkT", [P, KO, DKC], bf, kind="ExternalInput").ap(),
        "wvT": nc.dram_tensor("wvT", [P, KO, DKC], bf, kind="ExternalInput").ap(),
        "woT": nc.dram_tensor("woT", [DKC, D], bf, kind="ExternalInput").ap(),
        # em pretiled [b, h, qc, ki, ks, q]: 2 MB contiguous per (b,h,qc)
        "em": nc.dram_tensor("em", [B, HC, NQC, P, NKS, QC], bf,
                             kind="ExternalInput").ap(),
        "y": nc.dram_tensor("y", [T, D], bf, kind="ExternalOutput").ap(),
    }
    with tile.TileContext(nc) as tc:
        _build_body(nc, tc, io)
    nc.compile()
    return nc


_NC_CACHE = None


def _get_nc():
    global _NC_CACHE
    if _NC_CACHE is None:
        _NC_CACHE = build_nc()
    return _NC_CACHE


def _pretile_x(xT):
    """[D, T] -> [chunk, ki, ko, t] contiguous."""
    return np.ascontiguousarray(
        xT.reshape(KO, P, NCH, TC).transpose(2, 1, 0, 3)
    )


def make_in_maps(query, key, value, mask, rel_pos_bias,
                 Wq, bq, Wk, bk, Wv, bv, Wo, bo):
    """Host-side sharding/preprocessing -> per-core input dicts."""
    xqT = _pretile_x(query.reshape(T, D).T.astype(BF16))
    xkT = _pretile_x(key.reshape(T, D).T.astype(BF16))
    xvT = _pretile_x(value.reshape(T, D).T.astype(BF16))

    scale = 1.0 / np.sqrt(np.float32(DK))
    maskinv = (~mask[:, 0]).astype(np.float32)          # [B, Sq, Sk]

    # bq/bk handling: scores_full = (q+bq)(k+bk)^T * scale.
    # The (q'+bq)·bk term varies only along q => softmax-invariant, dropped.
    # The bq·(k'+bk) term varies along k; fold exp(delta_k) into em when
    # bq is nonzero (needs host k-projection).
    need_delta = bool(np.any(bq))
    if need_delta:
        k_proj = key.reshape(T, D).astype(np.float32) @ Wk.T.astype(np.float32) + bk

    in_maps = []
    for c in range(N_CORES):
        hs = slice(c * DKC, (c + 1) * DKC)
        wqT = np.ascontiguousarray(
            (Wq[hs, :] * scale).T.astype(BF16).reshape(KO, P, DKC).transpose(1, 0, 2))
        wkT = np.ascontiguousarray(
            Wk[hs, :].T.astype(BF16).reshape(KO, P, DKC).transpose(1, 0, 2))
        wvT = np.ascontiguousarray(
            Wv[hs, :].T.astype(BF16).reshape(KO, P, DKC).transpose(1, 0, 2))
        woT = np.ascontiguousarray(Wo[:, hs].T.astype(BF16))
        em = np.empty((B, HC, NQC, P, NKS, QC), dtype=BF16)
        for hi in range(HC):
            h = c * HC + hi
            ebT = np.exp(rel_pos_bias[0, h].astype(np.float32)).T  # [k, q]
            for b_ in range(B):
                ebb = ebT
                if need_delta:
                    delta = scale * (
                        k_proj[b_ * S:(b_ + 1) * S, h * DK:(h + 1) * DK]
                        @ bq[h * DK:(h + 1) * DK]
                        + np.dot(bq[h * DK:(h + 1) * DK], bk[h * DK:(h + 1) * DK])
                    )  # [S] along k
                    ebb = ebT * np.exp(delta)[:, None]
                emf = (ebb * maskinv[b_].T).astype(BF16)       # [k, q]
                # [k, q] -> [qc, ki, ks, q]
                em[b_, hi] = emf.reshape(NKS, P, NQC, QC).transpose(2, 1, 0, 3)
        in_maps.append({
            "xqT": xqT, "xkT": xkT, "xvT": xvT,
            "wqT": wqT, "wkT": wkT, "wvT": wvT, "woT": woT,
            "em": em,
        })
    return in_maps


def assemble_output(results, value_bias, Wo, bo):
    out = np.zeros((T, D), np.float32)
    for r in results:
        out += r["y"].astype(np.float32)
    # exact bv contribution: softmax rows sum to 1 => attn_out += bv,
    # so y += bv @ Wo^T; plus bo.
    out += value_bias.astype(np.float32) @ Wo.T.astype(np.float32)
    out += bo.astype(np.float32)[None, :]
    return out.reshape(B, S, D)


def kernel(query, key, value, mask, rel_pos_bias,
           Wq, bq, Wk, bk, Wv, bv, Wo, bo, _run_kwargs=None):
    query = np.asarray(query); key = np.asarray(key); value = np.asarray(value)
    mask = np.asarray(mask); rel_pos_bias = np.asarray(rel_pos_bias)
    Wq = np.asarray(Wq); Wk = np.asarray(Wk); Wv = np.asarray(Wv)
    Wo = np.asarray(Wo)
    bq = np.asarray(bq); bk = np.asarray(bk); bv = np.asarray(bv)
    bo = np.asarray(bo)

    nc = _get_nc()
    in_maps = make_in_maps(query, key, value, mask, rel_pos_bias,
                           Wq, bq, Wk, bk, Wv, bv, Wo, bo)
    kw = _run_kwargs or {}
    res = run_bass_kernel_spmd(nc, in_maps, core_ids=list(range(N_CORES)), **kw)
    out = assemble_output(res.results, bv, Wo, bo)
    if _run_kwargs is not None:
        kernel._last_results = res
    return out
